# revision 1
# baseline (speedup 1.0000x reference)
"""DistMult decoder kernel for 8 Trainium2 NeuronCores.

Computes out = (input1 * weight[type_index]) @ input2.T + bias with
input1 [8192, 512], input2 [8192, 512] in fp32, out [8192, 8192].

Sharding: rows of input1 (and thus rows of the output) are split across
the 8 cores; input2 / weight / bias are replicated. No communication.

Per-core device program (M = 1024 rows):
  - lhsT  [512, 1024]  = shard of input1, transposed on host (K-major)
  - rhs   [512, 8192]  = input2 transposed on host (K-major)
  - scale lhsT by w_r along K on device (DVE per-partition scalar mul)
  - GEMM: 16 n-tiles x 8 m-tiles x 4 k-matmuls accumulating in PSUM
  - PSUM -> SBUF copy + bias add on ACT, DMA out
"""

import os

import numpy as np

import concourse.bacc as bacc
import concourse.mybir as mybir
from concourse.bass_utils import run_bass_kernel_spmd
from concourse.tile import TileContext

N_CORES = 8
N1, N2, D = 8192, 8192, 512
M = N1 // N_CORES  # rows per core
P = 128            # partitions
KT = D // P        # 4 k-tiles
MT = M // P        # 8 m-tiles
NFREE = 512        # psum bank free size (fp32)
NT = N2 // NFREE   # 16 n-tiles

# test.py hooks: set TRACE=True before calling kernel() to profile; the
# BassKernelResults of the last run lands in LAST_RESULTS.
TRACE = os.environ.get("BASS_KERNEL_TRACE", "0") == "1"
LAST_RESULTS = None

_cached_nc = None


def _build():
    nc = bacc.Bacc(
        "TRN2", target_bir_lowering=False, debug=False, num_devices=N_CORES
    )
    lhsT = nc.dram_tensor("lhsT", [D, M], mybir.dt.float32, kind="ExternalInput")
    rhs = nc.dram_tensor("rhs", [D, N2], mybir.dt.float32, kind="ExternalInput")
    wr = nc.dram_tensor("wr", [P, KT], mybir.dt.float32, kind="ExternalInput")
    biasv = nc.dram_tensor("biasv", [P, 1], mybir.dt.float32, kind="ExternalInput")
    out = nc.dram_tensor("out", [M, N2], mybir.dt.float32, kind="ExternalOutput")

    f32 = mybir.dt.float32
    with TileContext(nc) as tc:
        with (
            tc.tile_pool(name="const", bufs=1) as constp,
            tc.tile_pool(name="lhs", bufs=1) as lhsp,
            tc.tile_pool(name="rhsp", bufs=3) as rhsp,
            tc.tile_pool(name="outp", bufs=8) as outp,
            tc.tile_pool(name="psum", bufs=8, space="PSUM") as psump,
        ):
            wr_t = constp.tile([P, KT], f32, tag="wr")
            nc.sync.dma_start(out=wr_t[:], in_=wr[:, :])
            bias_t = constp.tile([P, 1], f32, tag="bias")
            nc.sync.dma_start(out=bias_t[:], in_=biasv[:, :])

            # Resident lhsT k-tiles, scaled by w_r (w_r[k*128+p] = wr_t[p, k]).
            lhs_tiles = []
            for k in range(KT):
                t = lhsp.tile([P, M], f32, tag=f"lhs{k}")
                nc.sync.dma_start(out=t[:], in_=lhsT[k * P : (k + 1) * P, :])
                nc.vector.tensor_scalar_mul(t[:], t[:], wr_t[:, k : k + 1])
                lhs_tiles.append(t)

            for n in range(NT):
                rt = rhsp.tile([P, KT * NFREE], f32, tag="rhs")
                for k in range(KT):
                    nc.sync.dma_start(
                        out=rt[:, k * NFREE : (k + 1) * NFREE],
                        in_=rhs[k * P : (k + 1) * P, n * NFREE : (n + 1) * NFREE],
                    )
                for m in range(MT):
                    ps = psump.tile([P, NFREE], f32, tag="ps")
                    for k in range(KT):
                        nc.tensor.matmul(
                            ps[:],
                            lhs_tiles[k][:, m * P : (m + 1) * P],
                            rt[:, k * NFREE : (k + 1) * NFREE],
                            start=(k == 0),
                            stop=(k == KT - 1),
                        )
                    ot = outp.tile([P, NFREE], f32, tag="ot")
                    nc.scalar.activation(
                        ot[:],
                        ps[:],
                        mybir.ActivationFunctionType.Identity,
                        bias=bias_t[:, 0:1],
                    )
                    nc.sync.dma_start(
                        out=out[m * P : (m + 1) * P, n * NFREE : (n + 1) * NFREE],
                        in_=ot[:],
                    )
    nc.compile()
    return nc


def kernel(input1, input2, weight, bias, type_index):
    global _cached_nc, LAST_RESULTS

    input1 = np.ascontiguousarray(np.asarray(input1, dtype=np.float32))
    input2 = np.ascontiguousarray(np.asarray(input2, dtype=np.float32))
    weight = np.asarray(weight, dtype=np.float32)
    bias = np.asarray(bias, dtype=np.float32).reshape(-1)
    w_r = np.ascontiguousarray(weight[int(type_index)])  # [D]

    # K-major host layouts.
    rhsT = np.ascontiguousarray(input2.T)  # [D, N2], shared by all cores
    wr_mat = np.ascontiguousarray(w_r.reshape(KT, P).T)  # [P, KT]
    bias_vec = np.full((P, 1), float(bias[0]), dtype=np.float32)

    in_maps = []
    for c in range(N_CORES):
        shard = input1[c * M : (c + 1) * M]  # [M, D]
        in_maps.append(
            {
                "lhsT": np.ascontiguousarray(shard.T),  # [D, M]
                "rhs": rhsT,
                "wr": wr_mat,
                "biasv": bias_vec,
            }
        )

    if _cached_nc is None:
        _cached_nc = _build()

    res = run_bass_kernel_spmd(
        _cached_nc, in_maps, core_ids=list(range(N_CORES)), trace=TRACE
    )
    LAST_RESULTS = res
    return np.concatenate([res.results[c]["out"] for c in range(N_CORES)], axis=0)


# revision 2
# speedup vs baseline: 2.4673x; 2.4673x over previous
"""DistMult decoder kernel for 8 Trainium2 NeuronCores.

Computes out = (input1 * weight[type_index]) @ input2.T + bias with
input1 [8192, 512], input2 [8192, 512] in fp32, out [8192, 8192].

Sharding: rows of input1 (and thus rows of the output) are split across
the 8 cores; input2 / weight / bias are replicated. No communication.

Per-core device program (M = 1024 rows):
  - lhsT  [512, 1024]  = w_r-scaled shard of input1, transposed on host
  - rhs   [512, 8192]  = input2 transposed on host (K-major)
  - both fed to the PE as float32r (fp32 bit layout, 1 cycle/row at
    N=512 vs 4 for plain fp32; ~1.6e-4 matmul rel err vs 2.3e-3 bf16)
  - GEMM: 16 n-tiles x 8 m-tiles x 4 k-matmuls accumulating in PSUM
  - PSUM -> SBUF copy + bias add on ACT, DMA out
"""

import os

import numpy as np

import concourse.bacc as bacc
import concourse.mybir as mybir
from concourse.bass_utils import run_bass_kernel_spmd
from concourse.tile import TileContext

N_CORES = 8
N1, N2, D = 8192, 8192, 512
M = N1 // N_CORES  # rows per core
P = 128            # partitions
KT = D // P        # 4 k-tiles
MT = M // P        # 8 m-tiles
NFREE = 512        # psum bank free size (fp32)
NT = N2 // NFREE   # 16 n-tiles

# test.py hooks: set TRACE=True before calling kernel() to profile; the
# BassKernelResults of the last run lands in LAST_RESULTS.
TRACE = os.environ.get("BASS_KERNEL_TRACE", "0") == "1"
LAST_RESULTS = None

_cached_nc = None


def _build():
    nc = bacc.Bacc(
        "TRN2", target_bir_lowering=False, debug=False, num_devices=N_CORES
    )
    f32 = mybir.dt.float32
    f32r = mybir.dt.float32r
    lhsT = nc.dram_tensor("lhsT", [D, M], f32r, kind="ExternalInput")
    rhs = nc.dram_tensor("rhs", [D, N2], f32r, kind="ExternalInput")
    biasv = nc.dram_tensor("biasv", [P, 1], f32, kind="ExternalInput")
    out = nc.dram_tensor("out", [M, N2], f32, kind="ExternalOutput")

    with TileContext(nc) as tc:
        with (
            tc.tile_pool(name="const", bufs=1) as constp,
            tc.tile_pool(name="lhs", bufs=1) as lhsp,
            tc.tile_pool(name="rhsp", bufs=3) as rhsp,
            tc.tile_pool(name="outp", bufs=8) as outp,
            tc.tile_pool(name="psum", bufs=8, space="PSUM") as psump,
        ):
            bias_t = constp.tile([P, 1], f32, tag="bias")
            nc.sync.dma_start(out=bias_t[:], in_=biasv[:, :])

            # Resident lhsT k-tiles (pre-scaled by w_r on host).
            lhs_tiles = []
            for k in range(KT):
                t = lhsp.tile([P, M], f32r, tag=f"lhs{k}")
                nc.sync.dma_start(out=t[:], in_=lhsT[k * P : (k + 1) * P, :])
                lhs_tiles.append(t)

            for n in range(NT):
                rt = rhsp.tile([P, KT * NFREE], f32r, tag="rhs")
                for k in range(KT):
                    nc.sync.dma_start(
                        out=rt[:, k * NFREE : (k + 1) * NFREE],
                        in_=rhs[k * P : (k + 1) * P, n * NFREE : (n + 1) * NFREE],
                    )
                for m in range(MT):
                    ps = psump.tile([P, NFREE], f32, tag="ps")
                    for k in range(KT):
                        nc.tensor.matmul(
                            ps[:],
                            lhs_tiles[k][:, m * P : (m + 1) * P],
                            rt[:, k * NFREE : (k + 1) * NFREE],
                            start=(k == 0),
                            stop=(k == KT - 1),
                        )
                    ot = outp.tile([P, NFREE], f32, tag="ot")
                    nc.scalar.activation(
                        ot[:],
                        ps[:],
                        mybir.ActivationFunctionType.Identity,
                        bias=bias_t[:, 0:1],
                    )
                    nc.sync.dma_start(
                        out=out[m * P : (m + 1) * P, n * NFREE : (n + 1) * NFREE],
                        in_=ot[:],
                    )
    nc.compile()
    return nc


def kernel(input1, input2, weight, bias, type_index):
    global _cached_nc, LAST_RESULTS

    input1 = np.asarray(input1, dtype=np.float32)
    input2 = np.ascontiguousarray(np.asarray(input2, dtype=np.float32))
    weight = np.asarray(weight, dtype=np.float32)
    bias = np.asarray(bias, dtype=np.float32).reshape(-1)
    w_r = weight[int(type_index)]  # [D]

    # Host-side prep: fold the w_r row-scale into input1, lay both GEMM
    # operands out K-major so the device does contiguous DMA only.
    scaled = input1 * w_r[None, :]  # [N1, D]
    rhsT = np.ascontiguousarray(input2.T)  # [D, N2], shared by all cores
    bias_vec = np.full((P, 1), float(bias[0]), dtype=np.float32)

    in_maps = []
    for c in range(N_CORES):
        shard = scaled[c * M : (c + 1) * M]  # [M, D]
        in_maps.append(
            {
                "lhsT": np.ascontiguousarray(shard.T),  # [D, M]
                "rhs": rhsT,
                "biasv": bias_vec,
            }
        )

    if _cached_nc is None:
        _cached_nc = _build()

    res = run_bass_kernel_spmd(
        _cached_nc, in_maps, core_ids=list(range(N_CORES)), trace=TRACE
    )
    LAST_RESULTS = res
    return np.concatenate([res.results[c]["out"] for c in range(N_CORES)], axis=0)


# revision 4
# speedup vs baseline: 3.1269x; 1.2673x over previous
"""DistMult decoder kernel for 8 Trainium2 NeuronCores.

Computes out = (input1 * weight[type_index]) @ input2.T + bias with
input1 [8192, 512], input2 [8192, 512] in fp32, out [8192, 8192].

Sharding: rows of input1 (and thus rows of the output) are split across
the 8 cores; input2 / weight / bias are replicated. No communication.

Per-core device program (M = 1024 rows):
  - lhsT  [512, 1024]  = w_r-scaled shard of input1, transposed + cast
    to fp16 on host (K-major)
  - rhs   [512, 8192]  = input2 transposed + cast to fp16 on host
  - fp16 operands run the PE at 1 cycle/row (4x fp32) with fp32 PSUM
    accumulation; max-rel error vs the fp32 reference ~3e-4
  - GEMM over n-pairs: 8 groups x 8 m-tiles x (4 k x 2 n) matmuls
  - PSUM -> SBUF copy + bias add on ACT, 512 KB output stores
"""

import os

import numpy as np

import concourse.bacc as bacc
import concourse.mybir as mybir
from concourse.bass_utils import run_bass_kernel_spmd
from concourse.tile import TileContext

N_CORES = 8
N1, N2, D = 8192, 8192, 512
M = N1 // N_CORES  # rows per core
P = 128            # partitions
KT = D // P        # 4 k-tiles
MT = M // P        # 8 m-tiles
NFREE = 512        # psum bank free size (fp32)
NGRP = 1024        # n columns per group (pair of psum banks)
NT = N2 // NGRP    # 8 n-groups

# test.py hooks: set TRACE=True before calling kernel() to profile; the
# BassKernelResults of the last run lands in LAST_RESULTS.
TRACE = os.environ.get("BASS_KERNEL_TRACE", "0") == "1"
LAST_RESULTS = None

_cached_nc = None


def _build():
    nc = bacc.Bacc(
        "TRN2", target_bir_lowering=False, debug=False, num_devices=N_CORES
    )
    f32 = mybir.dt.float32
    f16 = mybir.dt.float16
    lhsT = nc.dram_tensor("lhsT", [D, M], f16, kind="ExternalInput")
    rhs = nc.dram_tensor("rhs", [D, N2], f16, kind="ExternalInput")
    biasv = nc.dram_tensor("biasv", [P, 1], f32, kind="ExternalInput")
    out = nc.dram_tensor("out", [M, N2], f32, kind="ExternalOutput")

    # K-major DRAM views split into [P, KT, cols] for single-DMA loads.
    lhsT_r = lhsT[:, :].rearrange("(kt p) m -> p kt m", p=P)
    rhs_r = rhs[:, :].rearrange("(kt p) n -> p kt n", p=P)

    with TileContext(nc) as tc:
        with (
            tc.tile_pool(name="const", bufs=1) as constp,
            tc.tile_pool(name="lhs", bufs=1) as lhsp,
            tc.tile_pool(name="rhsp", bufs=3) as rhsp,
            tc.tile_pool(name="outp", bufs=6) as outp,
            tc.tile_pool(name="psum", bufs=4, space="PSUM") as psump,
        ):
            bias_t = constp.tile([P, 1], f32, tag="bias")
            nc.sync.dma_start(out=bias_t[:], in_=biasv[:, :])

            # Resident lhsT (pre-scaled by w_r on host), one 1 MB DMA.
            lt = lhsp.tile([P, KT, M], f16, tag="lhs")
            nc.sync.dma_start(out=lt[:], in_=lhsT_r)

            for n in range(NT):
                rt = rhsp.tile([P, KT, NGRP], f16, tag="rhs")
                nc.sync.dma_start(
                    out=rt[:], in_=rhs_r[:, :, n * NGRP : (n + 1) * NGRP]
                )
                for m in range(MT):
                    ps0 = psump.tile([P, NFREE], f32, tag="ps0")
                    ps1 = psump.tile([P, NFREE], f32, tag="ps1")
                    for k in range(KT):
                        w = lt[:, k, m * P : (m + 1) * P]
                        nc.tensor.matmul(
                            ps0[:], w, rt[:, k, 0:NFREE],
                            start=(k == 0), stop=(k == KT - 1),
                        )
                        nc.tensor.matmul(
                            ps1[:], w, rt[:, k, NFREE:NGRP],
                            start=(k == 0), stop=(k == KT - 1),
                        )
                    ot = outp.tile([P, NGRP], f32, tag="ot")
                    nc.scalar.activation(
                        ot[:, 0:NFREE], ps0[:],
                        mybir.ActivationFunctionType.Identity,
                        bias=bias_t[:, 0:1],
                    )
                    nc.scalar.activation(
                        ot[:, NFREE:NGRP], ps1[:],
                        mybir.ActivationFunctionType.Identity,
                        bias=bias_t[:, 0:1],
                    )
                    nc.sync.dma_start(
                        out=out[m * P : (m + 1) * P, n * NGRP : (n + 1) * NGRP],
                        in_=ot[:],
                    )
    nc.compile()
    return nc


def kernel(input1, input2, weight, bias, type_index):
    global _cached_nc, LAST_RESULTS

    input1 = np.asarray(input1, dtype=np.float32)
    input2 = np.asarray(input2, dtype=np.float32)
    weight = np.asarray(weight, dtype=np.float32)
    bias = np.asarray(bias, dtype=np.float32).reshape(-1)
    w_r = weight[int(type_index)]  # [D]

    # Host-side prep: fold the w_r row-scale into input1, lay both GEMM
    # operands out K-major, cast to fp16 (device accumulates in fp32).
    scaled = input1 * w_r[None, :]  # [N1, D]
    rhsT = np.ascontiguousarray(input2.T.astype(np.float16))  # [D, N2]
    bias_vec = np.full((P, 1), float(bias[0]), dtype=np.float32)

    in_maps = []
    for c in range(N_CORES):
        shard = scaled[c * M : (c + 1) * M]  # [M, D]
        in_maps.append(
            {
                "lhsT": np.ascontiguousarray(shard.T.astype(np.float16)),
                "rhs": rhsT,
                "biasv": bias_vec,
            }
        )

    if _cached_nc is None:
        _cached_nc = _build()

    res = run_bass_kernel_spmd(
        _cached_nc, in_maps, core_ids=list(range(N_CORES)), trace=TRACE
    )
    LAST_RESULTS = res
    return np.concatenate([res.results[c]["out"] for c in range(N_CORES)], axis=0)


# revision 7
# speedup vs baseline: 3.2747x; 1.0473x over previous
"""DistMult decoder kernel for 8 Trainium2 NeuronCores.

Computes out = (input1 * weight[type_index]) @ input2.T + bias with
input1 [8192, 512], input2 [8192, 512] in fp32, out [8192, 8192].

Sharding: rows of input1 (and thus rows of the output) are split across
the 8 cores; input2 / weight / bias are replicated. No communication.

Per-core device program (M = 1024 rows):
  - lhsT  [512, 1024]  = w_r-scaled shard of input1, transposed + cast
    to fp16 on host (K-major)
  - rhs   [512, 8192]  = input2 transposed + cast to fp16 on host
  - fp16 operands run the PE at 1 cycle/row (4x fp32) with fp32 PSUM
    accumulation; max-rel error vs the fp32 reference ~3e-4
  - GEMM over n-pairs: 8 groups x 8 m-tiles x (4 k x 2 n) matmuls
  - PSUM -> SBUF copy + bias add on ACT, 512 KB output stores
"""

import os

import numpy as np

import concourse.bacc as bacc
import concourse.mybir as mybir
from concourse.bass_utils import run_bass_kernel_spmd
from concourse.tile import TileContext

N_CORES = 8
N1, N2, D = 8192, 8192, 512
M = N1 // N_CORES  # rows per core
P = 128            # partitions
KT = D // P        # 4 k-tiles
MT = M // P        # 8 m-tiles
NFREE = 512        # psum bank free size (fp32)
NGRP = 1024        # n columns per group (pair of psum banks)
NT = N2 // NGRP    # 8 n-groups

# test.py hooks: set TRACE=True before calling kernel() to profile; the
# BassKernelResults of the last run lands in LAST_RESULTS.
TRACE = os.environ.get("BASS_KERNEL_TRACE", "0") == "1"
LAST_RESULTS = None

_cached_nc = None


def _build():
    nc = bacc.Bacc(
        "TRN2", target_bir_lowering=False, debug=False, num_devices=N_CORES
    )
    f32 = mybir.dt.float32
    f16 = mybir.dt.float16
    lhsT = nc.dram_tensor("lhsT", [D, M], f16, kind="ExternalInput")
    rhs = nc.dram_tensor("rhs", [D, N2], f16, kind="ExternalInput")
    biasv = nc.dram_tensor("biasv", [P, 1], f32, kind="ExternalInput")
    out = nc.dram_tensor("out", [M, N2], f32, kind="ExternalOutput")

    # K-major DRAM views split into [P, KT, cols] for single-DMA loads.
    lhsT_r = lhsT[:, :].rearrange("(kt p) m -> p kt m", p=P)
    rhs_r = rhs[:, :].rearrange("(kt p) n -> p kt n", p=P)

    with TileContext(nc) as tc:
        with (
            tc.tile_pool(name="const", bufs=1) as constp,
            tc.tile_pool(name="lhs", bufs=1) as lhsp,
            tc.tile_pool(name="rhsp", bufs=4) as rhsp,
            tc.tile_pool(name="outp", bufs=6) as outp,
            tc.tile_pool(name="psum", bufs=4, space="PSUM") as psump,
        ):
            bias_t = constp.tile([P, 1], f32, tag="bias")
            nc.sync.dma_start(out=bias_t[:], in_=biasv[:, :])

            # Resident lhsT (pre-scaled by w_r on host), split per k-tile
            # so the first matmuls can start before the full 1 MB lands.
            lt = lhsp.tile([P, KT, M], f16, tag="lhs")
            for k in range(KT):
                nc.sync.dma_start(out=lt[:, k, :], in_=lhsT_r[:, k, :])

            # rhs loads run on the GpSimd (SWDGE) queue so they never sit
            # behind output stores in the Sync engine's FIFO; PF groups of
            # explicit prefetch keep the PE fed across group boundaries.
            PF = 2
            rts = {}

            def load_rhs(g, engine):
                rt = rhsp.tile([P, KT, NGRP], f16, tag="rhs")
                engine.dma_start(
                    out=rt[:], in_=rhs_r[:, :, g * NGRP : (g + 1) * NGRP]
                )
                rts[g] = rt

            # First group on the low-latency HWDGE path, halves for a
            # faster first matmul.
            rt0 = rhsp.tile([P, KT, NGRP], f16, tag="rhs")
            nc.sync.dma_start(out=rt0[:, :, 0:NFREE], in_=rhs_r[:, :, 0:NFREE])
            nc.sync.dma_start(
                out=rt0[:, :, NFREE:NGRP], in_=rhs_r[:, :, NFREE:NGRP]
            )
            rts[0] = rt0
            for g in range(1, 1 + PF):
                load_rhs(g, nc.gpsimd)

            for n in range(NT):
                rt = rts.pop(n)
                if n + PF + 1 < NT:
                    load_rhs(n + PF + 1, nc.gpsimd)
                for m in range(MT):
                    ps0 = psump.tile([P, NFREE], f32, tag="ps0")
                    ps1 = psump.tile([P, NFREE], f32, tag="ps1")
                    for k in range(KT):
                        w = lt[:, k, m * P : (m + 1) * P]
                        nc.tensor.matmul(
                            ps0[:], w, rt[:, k, 0:NFREE],
                            start=(k == 0), stop=(k == KT - 1),
                        )
                        nc.tensor.matmul(
                            ps1[:], w, rt[:, k, NFREE:NGRP],
                            start=(k == 0), stop=(k == KT - 1),
                        )
                    ot = outp.tile([P, NGRP], f32, tag="ot")
                    # Split psum->sbuf+bias between ACT and the otherwise
                    # idle DVE so neither serializes the psum pool.
                    nc.scalar.activation(
                        ot[:, 0:NFREE], ps0[:],
                        mybir.ActivationFunctionType.Identity,
                        bias=bias_t[:, 0:1],
                    )
                    nc.vector.tensor_scalar_add(
                        ot[:, NFREE:NGRP], ps1[:], bias_t[:, 0:1]
                    )
                    nc.sync.dma_start(
                        out=out[m * P : (m + 1) * P, n * NGRP : (n + 1) * NGRP],
                        in_=ot[:],
                    )
    nc.compile()
    return nc


def kernel(input1, input2, weight, bias, type_index):
    global _cached_nc, LAST_RESULTS

    input1 = np.asarray(input1, dtype=np.float32)
    input2 = np.asarray(input2, dtype=np.float32)
    weight = np.asarray(weight, dtype=np.float32)
    bias = np.asarray(bias, dtype=np.float32).reshape(-1)
    w_r = weight[int(type_index)]  # [D]

    # Host-side prep: fold the w_r row-scale into input1, lay both GEMM
    # operands out K-major, cast to fp16 (device accumulates in fp32).
    scaled = input1 * w_r[None, :]  # [N1, D]
    rhsT = np.ascontiguousarray(input2.T.astype(np.float16))  # [D, N2]
    bias_vec = np.full((P, 1), float(bias[0]), dtype=np.float32)

    in_maps = []
    for c in range(N_CORES):
        shard = scaled[c * M : (c + 1) * M]  # [M, D]
        in_maps.append(
            {
                "lhsT": np.ascontiguousarray(shard.T.astype(np.float16)),
                "rhs": rhsT,
                "biasv": bias_vec,
            }
        )

    if _cached_nc is None:
        _cached_nc = _build()

    res = run_bass_kernel_spmd(
        _cached_nc, in_maps, core_ids=list(range(N_CORES)), trace=TRACE
    )
    LAST_RESULTS = res
    return np.concatenate([res.results[c]["out"] for c in range(N_CORES)], axis=0)


# revision 10
# speedup vs baseline: 3.3062x; 1.0096x over previous
"""DistMult decoder kernel for 8 Trainium2 NeuronCores.

Computes out = (input1 * weight[type_index]) @ input2.T + bias with
input1 [8192, 512], input2 [8192, 512] in fp32, out [8192, 8192].

Sharding: rows of input1 (and thus rows of the output) are split across
the 8 cores; input2 / weight / bias are replicated. No communication.

Per-core device program (M = 1024 rows):
  - lhsT  [512, 1024]  = w_r-scaled shard of input1, transposed + cast
    to fp16 on host (K-major)
  - rhs   [512, 8192]  = input2 transposed + cast to fp16 on host
  - fp16 operands run the PE at 1 cycle/row (4x fp32) with fp32 PSUM
    accumulation; max-rel error vs the fp32 reference ~3e-4
  - GEMM over n-pairs: 8 groups x 8 m-tiles x (4 k x 2 n) matmuls
  - PSUM -> SBUF copy + bias add on ACT, 512 KB output stores
"""

import os

import numpy as np

import concourse.bacc as bacc
import concourse.mybir as mybir
from concourse.bass_utils import run_bass_kernel_spmd
from concourse.tile import TileContext

N_CORES = 8
N1, N2, D = 8192, 8192, 512
M = N1 // N_CORES  # rows per core
P = 128            # partitions
KT = D // P        # 4 k-tiles
MT = M // P        # 8 m-tiles
NFREE = 512        # psum bank free size (fp32)
NGRP = 1024        # n columns per group (pair of psum banks)
NT = N2 // NGRP    # 8 n-groups

# test.py hooks: set TRACE=True before calling kernel() to profile; the
# BassKernelResults of the last run lands in LAST_RESULTS.
TRACE = os.environ.get("BASS_KERNEL_TRACE", "0") == "1"
LAST_RESULTS = None

_cached_nc = None


def _build():
    nc = bacc.Bacc(
        "TRN2", target_bir_lowering=False, debug=False, enable_asserts=False, num_devices=N_CORES
    )
    f32 = mybir.dt.float32
    f16 = mybir.dt.float16
    lhsT = nc.dram_tensor("lhsT", [D, M], f16, kind="ExternalInput")
    rhs = nc.dram_tensor("rhs", [D, N2], f16, kind="ExternalInput")
    biasv = nc.dram_tensor("biasv", [P, 1], f32, kind="ExternalInput")
    out = nc.dram_tensor("out", [M, N2], f32, kind="ExternalOutput")

    # K-major DRAM views split into [P, KT, cols] for single-DMA loads.
    lhsT_r = lhsT[:, :].rearrange("(kt p) m -> p kt m", p=P)
    rhs_r = rhs[:, :].rearrange("(kt p) n -> p kt n", p=P)

    with TileContext(nc) as tc:
        with (
            tc.tile_pool(name="const", bufs=1) as constp,
            tc.tile_pool(name="lhs", bufs=1) as lhsp,
            tc.tile_pool(name="rhsp", bufs=4) as rhsp,
            tc.tile_pool(name="outp", bufs=6) as outp,
            tc.tile_pool(name="psum", bufs=4, space="PSUM") as psump,
        ):
            # Head: only the data the first matmuls need — the first rhs
            # half-group and the first lhsT k-tile — goes out up front, so
            # the PE starts ~2.5 us after the engine preamble instead of
            # waiting behind 5 MB of prefetch. Everything else streams in
            # while group 0 computes.
            lt = lhsp.tile([P, KT, M], f16, tag="lhs")
            rt0 = rhsp.tile([P, KT, NGRP], f16, tag="rhs")
            nc.sync.dma_start(out=rt0[:, :, 0:NFREE], in_=rhs_r[:, :, 0:NFREE])
            nc.sync.dma_start(out=lt[:, 0, :], in_=lhsT_r[:, 0, :])
            bias_t = constp.tile([P, 1], f32, tag="bias")
            nc.sync.dma_start(out=bias_t[:], in_=biasv[:, :])
            nc.sync.dma_start(
                out=rt0[:, :, NFREE:NGRP], in_=rhs_r[:, :, NFREE:NGRP]
            )
            for k in range(1, KT):
                nc.sync.dma_start(out=lt[:, k, :], in_=lhsT_r[:, k, :])

            # rhs loads run on the GpSimd (SWDGE) queue so they never sit
            # behind output stores in the Sync engine's FIFO; one group of
            # lookahead keeps the PE fed across group boundaries.
            rts = {0: rt0}

            def load_rhs(g):
                rt = rhsp.tile([P, KT, NGRP], f16, tag="rhs")
                nc.gpsimd.dma_start(
                    out=rt[:], in_=rhs_r[:, :, g * NGRP : (g + 1) * NGRP]
                )
                rts[g] = rt

            for n in range(NT):
                rt = rts.pop(n)
                for m in range(MT):
                    # Stagger rhs prefetch into the compute stream, keeping
                    # two groups of lookahead: iter 0 loads groups 1 and 2
                    # (staggered), iter n>=1 tops up with group n+2.
                    if m == 0:
                        if n == 0:
                            load_rhs(1)
                        elif n + 2 < NT:
                            load_rhs(n + 2)
                    if m == 4 and n == 0:
                        load_rhs(2)
                    ps0 = psump.tile([P, NFREE], f32, tag="ps0")
                    ps1 = psump.tile([P, NFREE], f32, tag="ps1")
                    for k in range(KT):
                        w = lt[:, k, m * P : (m + 1) * P]
                        nc.tensor.matmul(
                            ps0[:], w, rt[:, k, 0:NFREE],
                            start=(k == 0), stop=(k == KT - 1),
                        )
                        nc.tensor.matmul(
                            ps1[:], w, rt[:, k, NFREE:NGRP],
                            start=(k == 0), stop=(k == KT - 1),
                        )
                    ot = outp.tile([P, NGRP], f32, tag="ot")
                    # Split psum->sbuf+bias between ACT and the otherwise
                    # idle DVE so neither serializes the psum pool.
                    nc.scalar.activation(
                        ot[:, 0:NFREE], ps0[:],
                        mybir.ActivationFunctionType.Identity,
                        bias=bias_t[:, 0:1],
                    )
                    nc.vector.tensor_scalar_add(
                        ot[:, NFREE:NGRP], ps1[:], bias_t[:, 0:1]
                    )
                    nc.sync.dma_start(
                        out=out[m * P : (m + 1) * P, n * NGRP : (n + 1) * NGRP],
                        in_=ot[:],
                    )
    nc.compile()
    return nc


def kernel(input1, input2, weight, bias, type_index):
    global _cached_nc, LAST_RESULTS

    input1 = np.asarray(input1, dtype=np.float32)
    input2 = np.asarray(input2, dtype=np.float32)
    weight = np.asarray(weight, dtype=np.float32)
    bias = np.asarray(bias, dtype=np.float32).reshape(-1)
    w_r = weight[int(type_index)]  # [D]

    # Host-side prep: fold the w_r row-scale into input1, lay both GEMM
    # operands out K-major, cast to fp16 (device accumulates in fp32).
    scaled = input1 * w_r[None, :]  # [N1, D]
    rhsT = np.ascontiguousarray(input2.T.astype(np.float16))  # [D, N2]
    bias_vec = np.full((P, 1), float(bias[0]), dtype=np.float32)

    in_maps = []
    for c in range(N_CORES):
        shard = scaled[c * M : (c + 1) * M]  # [M, D]
        in_maps.append(
            {
                "lhsT": np.ascontiguousarray(shard.T.astype(np.float16)),
                "rhs": rhsT,
                "biasv": bias_vec,
            }
        )

    if _cached_nc is None:
        _cached_nc = _build()

    res = run_bass_kernel_spmd(
        _cached_nc, in_maps, core_ids=list(range(N_CORES)), trace=TRACE
    )
    LAST_RESULTS = res
    return np.concatenate([res.results[c]["out"] for c in range(N_CORES)], axis=0)


# revision 12
# speedup vs baseline: 3.3197x; 1.0041x over previous
"""DistMult decoder kernel for 8 Trainium2 NeuronCores.

Computes out = (input1 * weight[type_index]) @ input2.T + bias with
input1 [8192, 512], input2 [8192, 512] in fp32, out [8192, 8192].

Sharding: rows of input1 (and thus rows of the output) are split across
the 8 cores; input2 / weight / bias are replicated. No communication.

Per-core device program (M = 1024 rows):
  - lhsT  [512, 1024]  = w_r-scaled shard of input1, transposed + cast
    to fp16 on host (K-major)
  - rhs   [512, 8192]  = input2 transposed + cast to fp16 on host
  - fp16 operands run the PE at 1 cycle/row (4x fp32) with fp32 PSUM
    accumulation; max-rel error vs the fp32 reference ~3e-4
  - GEMM over n-pairs: 8 groups x 8 m-tiles x (4 k x 2 n) matmuls
  - PSUM -> SBUF copy + bias add on ACT, 512 KB output stores
"""

import os

import numpy as np

import concourse.bacc as bacc
import concourse.mybir as mybir
from concourse.bass_utils import run_bass_kernel_spmd
from concourse.tile import TileContext

N_CORES = 8
N1, N2, D = 8192, 8192, 512
M = N1 // N_CORES  # rows per core
P = 128            # partitions
KT = D // P        # 4 k-tiles
MT = M // P        # 8 m-tiles
NFREE = 512        # psum bank free size (fp32)
NGRP = 1024        # n columns per group (pair of psum banks)
NT = N2 // NGRP    # 8 n-groups

# test.py hooks: set TRACE=True before calling kernel() to profile; the
# BassKernelResults of the last run lands in LAST_RESULTS.
TRACE = os.environ.get("BASS_KERNEL_TRACE", "0") == "1"
LAST_RESULTS = None

_cached_nc = None


def _build():
    nc = bacc.Bacc(
        "TRN2", target_bir_lowering=False, debug=False, enable_asserts=False, num_devices=N_CORES
    )
    f32 = mybir.dt.float32
    f16 = mybir.dt.float16
    lhsT = nc.dram_tensor("lhsT", [D, M], f16, kind="ExternalInput")
    rhs = nc.dram_tensor("rhs", [D, N2], f16, kind="ExternalInput")
    biasv = nc.dram_tensor("biasv", [P, 1], f32, kind="ExternalInput")
    out = nc.dram_tensor("out", [M, N2], f32, kind="ExternalOutput")

    # K-major DRAM views split into [P, KT, cols] for single-DMA loads.
    lhsT_r = lhsT[:, :].rearrange("(kt p) m -> p kt m", p=P)
    rhs_r = rhs[:, :].rearrange("(kt p) n -> p kt n", p=P)

    with TileContext(nc) as tc:
        with (
            tc.tile_pool(name="const", bufs=1) as constp,
            tc.tile_pool(name="lhs", bufs=1) as lhsp,
            tc.tile_pool(name="rhsp", bufs=4) as rhsp,
            tc.tile_pool(name="outp", bufs=6) as outp,
            tc.tile_pool(name="psum", bufs=4, space="PSUM") as psump,
        ):
            # Head: spread the startup loads across all three DGE rings
            # (each ring tops out well below HBM bandwidth) so the PE can
            # start as soon as the preamble ends: Sync carries the first
            # rhs half-group, the Scalar ring carries lhsT, GpSimd carries
            # the second rhs half-group.
            lt = lhsp.tile([P, KT, M], f16, tag="lhs")
            rt0 = rhsp.tile([P, KT, NGRP], f16, tag="rhs")
            nc.sync.dma_start(out=rt0[:, :, 0:NFREE], in_=rhs_r[:, :, 0:NFREE])
            for k in range(KT):
                nc.scalar.dma_start(out=lt[:, k, :], in_=lhsT_r[:, k, :])
            bias_t = constp.tile([P, 1], f32, tag="bias")
            nc.sync.dma_start(out=bias_t[:], in_=biasv[:, :])
            nc.gpsimd.dma_start(
                out=rt0[:, :, NFREE:NGRP], in_=rhs_r[:, :, NFREE:NGRP]
            )

            # rhs loads run on the GpSimd (SWDGE) queue so they never sit
            # behind output stores in the Sync engine's FIFO; one group of
            # lookahead keeps the PE fed across group boundaries.
            rts = {0: rt0}

            def load_rhs(g):
                rt = rhsp.tile([P, KT, NGRP], f16, tag="rhs")
                nc.gpsimd.dma_start(
                    out=rt[:], in_=rhs_r[:, :, g * NGRP : (g + 1) * NGRP]
                )
                rts[g] = rt

            for n in range(NT):
                rt = rts.pop(n)
                for m in range(MT):
                    # Stagger rhs prefetch into the compute stream, keeping
                    # two groups of lookahead: iter 0 loads groups 1 and 2
                    # (staggered), iter n>=1 tops up with group n+2.
                    if m == 0:
                        if n == 0:
                            load_rhs(1)
                        elif n + 2 < NT:
                            load_rhs(n + 2)
                    if m == 4 and n == 0:
                        load_rhs(2)
                    ps0 = psump.tile([P, NFREE], f32, tag="ps0")
                    ps1 = psump.tile([P, NFREE], f32, tag="ps1")
                    # ps0's k-loop completes before ps1 starts: the copy of
                    # ps0 can begin 3 matmuls earlier, and at kernel start
                    # the PE only waits on the first rhs half-group.
                    for k in range(KT):
                        nc.tensor.matmul(
                            ps0[:], lt[:, k, m * P : (m + 1) * P],
                            rt[:, k, 0:NFREE],
                            start=(k == 0), stop=(k == KT - 1),
                        )
                    for k in range(KT):
                        nc.tensor.matmul(
                            ps1[:], lt[:, k, m * P : (m + 1) * P],
                            rt[:, k, NFREE:NGRP],
                            start=(k == 0), stop=(k == KT - 1),
                        )
                    ot = outp.tile([P, NGRP], f32, tag="ot")
                    # Split psum->sbuf+bias between ACT and the otherwise
                    # idle DVE so neither serializes the psum pool.
                    nc.scalar.activation(
                        ot[:, 0:NFREE], ps0[:],
                        mybir.ActivationFunctionType.Identity,
                        bias=bias_t[:, 0:1],
                    )
                    nc.vector.tensor_scalar_add(
                        ot[:, NFREE:NGRP], ps1[:], bias_t[:, 0:1]
                    )
                    nc.sync.dma_start(
                        out=out[m * P : (m + 1) * P, n * NGRP : (n + 1) * NGRP],
                        in_=ot[:],
                    )
    nc.compile()
    return nc


def kernel(input1, input2, weight, bias, type_index):
    global _cached_nc, LAST_RESULTS

    input1 = np.asarray(input1, dtype=np.float32)
    input2 = np.asarray(input2, dtype=np.float32)
    weight = np.asarray(weight, dtype=np.float32)
    bias = np.asarray(bias, dtype=np.float32).reshape(-1)
    w_r = weight[int(type_index)]  # [D]

    # Host-side prep: fold the w_r row-scale into input1, lay both GEMM
    # operands out K-major, cast to fp16 (device accumulates in fp32).
    scaled = input1 * w_r[None, :]  # [N1, D]
    rhsT = np.ascontiguousarray(input2.T.astype(np.float16))  # [D, N2]
    bias_vec = np.full((P, 1), float(bias[0]), dtype=np.float32)

    in_maps = []
    for c in range(N_CORES):
        shard = scaled[c * M : (c + 1) * M]  # [M, D]
        in_maps.append(
            {
                "lhsT": np.ascontiguousarray(shard.T.astype(np.float16)),
                "rhs": rhsT,
                "biasv": bias_vec,
            }
        )

    if _cached_nc is None:
        _cached_nc = _build()

    res = run_bass_kernel_spmd(
        _cached_nc, in_maps, core_ids=list(range(N_CORES)), trace=TRACE
    )
    LAST_RESULTS = res
    return np.concatenate([res.results[c]["out"] for c in range(N_CORES)], axis=0)


# revision 14
# speedup vs baseline: 3.3881x; 1.0206x over previous
"""DistMult decoder kernel for 8 Trainium2 NeuronCores.

Computes out = (input1 * weight[type_index]) @ input2.T + bias with
input1 [8192, 512], input2 [8192, 512] in fp32, out [8192, 8192].

Sharding: rows of input1 (and thus rows of the output) are split across
the 8 cores; input2 / weight / bias are replicated. No communication.

Per-core device program (M = 1024 rows):
  - lhsT  [512, 1024]  = w_r-scaled shard of input1, transposed + cast
    to fp16 on host (K-major)
  - rhs   [512, 8192]  = input2 transposed + cast to fp16 on host
  - fp16 operands run the PE at 1 cycle/row (4x fp32) with fp32 PSUM
    accumulation; max-rel error vs the fp32 reference ~3e-4
  - GEMM over n-pairs: 8 groups x 8 m-tiles x (4 k x 2 n) matmuls
  - PSUM -> SBUF copy + bias add on ACT, 512 KB output stores
"""

import os

import numpy as np

import concourse.bacc as bacc
import concourse.mybir as mybir
from concourse.bass_utils import run_bass_kernel_spmd
from concourse.tile import TileContext

N_CORES = 8
N1, N2, D = 8192, 8192, 512
M = N1 // N_CORES  # rows per core
P = 128            # partitions
KT = D // P        # 4 k-tiles
MT = M // P        # 8 m-tiles
NFREE = 512        # psum bank free size (fp32)
NGRP = 1024        # n columns per group (pair of psum banks)
NT = N2 // NGRP    # 8 n-groups

# test.py hooks: set TRACE=True before calling kernel() to profile; the
# BassKernelResults of the last run lands in LAST_RESULTS.
TRACE = os.environ.get("BASS_KERNEL_TRACE", "0") == "1"
LAST_RESULTS = None

_cached_nc = None


def _build():
    nc = bacc.Bacc(
        "TRN2", target_bir_lowering=False, debug=False, enable_asserts=False, num_devices=N_CORES
    )
    f32 = mybir.dt.float32
    f16 = mybir.dt.float16
    lhsT = nc.dram_tensor("lhsT", [D, M], f16, kind="ExternalInput")
    rhs = nc.dram_tensor("rhs", [D, N2], f16, kind="ExternalInput")
    biasv = nc.dram_tensor("biasv", [P, 1], f32, kind="ExternalInput")
    out = nc.dram_tensor("out", [M, N2], f32, kind="ExternalOutput")

    # K-major DRAM views split into [P, KT, cols] for single-DMA loads.
    lhsT_r = lhsT[:, :].rearrange("(kt p) m -> p kt m", p=P)
    rhs_r = rhs[:, :].rearrange("(kt p) n -> p kt n", p=P)

    with TileContext(nc) as tc:
        with (
            tc.tile_pool(name="const", bufs=1) as constp,
            tc.tile_pool(name="lhs", bufs=1) as lhsp,
            tc.tile_pool(name="rhsp", bufs=4) as rhsp,
            tc.tile_pool(name="outp", bufs=8) as outp,
            tc.tile_pool(name="psum", bufs=4, space="PSUM") as psump,
        ):
            # Head: spread the startup loads across all three DGE rings
            # (each ring tops out well below HBM bandwidth) so the PE can
            # start as soon as the preamble ends: Sync carries the first
            # rhs half-group, the Scalar ring carries lhsT, GpSimd carries
            # the second rhs half-group.
            lt = lhsp.tile([P, KT, M], f16, tag="lhs")
            rt0 = rhsp.tile([P, KT, NGRP], f16, tag="rhs")
            nc.sync.dma_start(out=rt0[:, :, 0:NFREE], in_=rhs_r[:, :, 0:NFREE])
            for k in range(KT):
                eng = nc.scalar if k % 2 == 0 else nc.sync
                eng.dma_start(out=lt[:, k, :], in_=lhsT_r[:, k, :])
            bias_t = constp.tile([P, 1], f32, tag="bias")
            nc.scalar.dma_start(out=bias_t[:], in_=biasv[:, :])
            nc.gpsimd.dma_start(
                out=rt0[:, :, NFREE:NGRP], in_=rhs_r[:, :, NFREE:NGRP]
            )

            # rhs loads run on the GpSimd (SWDGE) queue so they never sit
            # behind output stores in the Sync engine's FIFO; one group of
            # lookahead keeps the PE fed across group boundaries.
            rts = {0: rt0}

            def load_rhs(g):
                rt = rhsp.tile([P, KT, NGRP], f16, tag="rhs")
                nc.gpsimd.dma_start(
                    out=rt[:], in_=rhs_r[:, :, g * NGRP : (g + 1) * NGRP]
                )
                rts[g] = rt

            for n in range(NT):
                rt = rts.pop(n)
                for m in range(MT):
                    # Stagger rhs prefetch into the compute stream, keeping
                    # two groups of lookahead: iter 0 loads groups 1 and 2
                    # (staggered), iter n>=1 tops up with group n+2.
                    if m == 0:
                        if n == 0:
                            load_rhs(1)
                        elif n + 2 < NT:
                            load_rhs(n + 2)
                    if m == 4 and n == 0:
                        load_rhs(2)
                    ps0 = psump.tile([P, NFREE], f32, tag="ps0")
                    ps1 = psump.tile([P, NFREE], f32, tag="ps1")
                    # ps0's k-loop completes before ps1 starts: the copy of
                    # ps0 can begin 3 matmuls earlier, and at kernel start
                    # the PE only waits on the first rhs half-group.
                    for k in range(KT):
                        nc.tensor.matmul(
                            ps0[:], lt[:, k, m * P : (m + 1) * P],
                            rt[:, k, 0:NFREE],
                            start=(k == 0), stop=(k == KT - 1),
                        )
                    for k in range(KT):
                        nc.tensor.matmul(
                            ps1[:], lt[:, k, m * P : (m + 1) * P],
                            rt[:, k, NFREE:NGRP],
                            start=(k == 0), stop=(k == KT - 1),
                        )
                    ot = outp.tile([P, NGRP], f32, tag="ot")
                    # Split psum->sbuf+bias between ACT and the otherwise
                    # idle DVE so neither serializes the psum pool.
                    nc.scalar.activation(
                        ot[:, 0:NFREE], ps0[:],
                        mybir.ActivationFunctionType.Identity,
                        bias=bias_t[:, 0:1],
                    )
                    nc.vector.tensor_scalar_add(
                        ot[:, NFREE:NGRP], ps1[:], bias_t[:, 0:1]
                    )
                    nc.sync.dma_start(
                        out=out[m * P : (m + 1) * P, n * NGRP : (n + 1) * NGRP],
                        in_=ot[:],
                    )
    nc.compile()
    return nc


def kernel(input1, input2, weight, bias, type_index):
    global _cached_nc, LAST_RESULTS

    input1 = np.asarray(input1, dtype=np.float32)
    input2 = np.asarray(input2, dtype=np.float32)
    weight = np.asarray(weight, dtype=np.float32)
    bias = np.asarray(bias, dtype=np.float32).reshape(-1)
    w_r = weight[int(type_index)]  # [D]

    # Host-side prep: fold the w_r row-scale into input1, lay both GEMM
    # operands out K-major, cast to fp16 (device accumulates in fp32).
    scaled = input1 * w_r[None, :]  # [N1, D]
    rhsT = np.ascontiguousarray(input2.T.astype(np.float16))  # [D, N2]
    bias_vec = np.full((P, 1), float(bias[0]), dtype=np.float32)

    in_maps = []
    for c in range(N_CORES):
        shard = scaled[c * M : (c + 1) * M]  # [M, D]
        in_maps.append(
            {
                "lhsT": np.ascontiguousarray(shard.T.astype(np.float16)),
                "rhs": rhsT,
                "biasv": bias_vec,
            }
        )

    if _cached_nc is None:
        _cached_nc = _build()

    res = run_bass_kernel_spmd(
        _cached_nc, in_maps, core_ids=list(range(N_CORES)), trace=TRACE
    )
    LAST_RESULTS = res
    return np.concatenate([res.results[c]["out"] for c in range(N_CORES)], axis=0)
